# revision 19
# baseline (speedup 1.0000x reference)
"""Self-contained GAT (PyG GATConv, concat=False) Bass/Tile kernel for 8
Trainium2 NeuronCores.  kernel(**inputs) takes the full-graph inputs and
returns the full [N, 32] output.

Strategy (dst-partition): nodes are sorted by in-degree (self-loops included)
and packed into groups of 128; inside a group the PARTITION index is the
destination node, and the free dim holds that node's incoming edges
(T = group max degree slots per node).  This removes the one-hot scatter
matrices entirely: segment-sum over a group's edges is T identity-matmuls
accumulated in PSUM.  Groups are snake-dealt to the 8 cores; all cores run
one shared program (per-group T schedule identical), per-core differences
live in the data (a per-core node permutation makes each core's own groups
occupy rows [g*128,(g+1)*128) of its private `ha` scratch).

Per core:
  Phase A: ha[r] = [h_cmajor(256) | a_src(8) | a_dst(8)] fp16 for ALL rows
           (x pre-transposed+fp16 on host, one 272-col fp16 matmul per
           128-row tile; PSUM->SBUF cast copies split ACT/DVE).
  Phase B per group: one multi-row indirect DMA gathers all T*128 edge
           source rows (264 cols); a_dst of the group's own nodes comes from
           a strided slice load; attention = exp(leaky(a_src+a_dst) - 8)
           computed in-place (DVE leaky, ACT exp, shift avoids fp16
           overflow; softmax is shift-invariant so no max pass is needed);
           messages multiply in fp16 2x mode (head-minor layout keeps the
           broadcast packed); T identity-matmuls accumulate [128,264] in
           PSUM (256 msg cols + 8 denominator cols); normalize + head-mean
           + bias on DVE; store [128,32].
Empty edge slots gather a pad row with a_src=-60000 so their exp underflows
to exactly 0 in fp16.
"""

import math

import numpy as np

import concourse.tile as tile
import concourse.mybir as mb
from concourse import bass, mybir

P = 128
F32 = mybir.dt.float32
F16 = mybir.dt.float16
I32 = mybir.dt.int32

HEADS = 8
OUT_C = 32
HC = HEADS * OUT_C          # 256
ROW = HC + 2 * HEADS        # 272 per ha row: h(c-major) | a_src | a_dst
GROW = HC + HEADS           # 264 gathered cols per edge row
NEG_SLOPE = 0.2
EXP_SHIFT = -8.0            # exp(s - 8): keeps fp16 in range; shift-invariant
DENOM_EPS = 1e-30
PAD_ASRC = -60000.0         # pad-slot a_src: exp underflows to exact 0
N_CORES = 8
CHUNK = 4                   # phase-A node tiles per DMA chunk
GCHUNK = 1                  # edge slots per indirect gather: the walrus
                            # lowering only handles [P,1] offset APs


# ----------------------------------------------------------------------------
# Host-side planning
# ----------------------------------------------------------------------------

def plan_dst(edge_index: np.ndarray, n_nodes: int, n_cores: int = N_CORES):
    src = np.asarray(edge_index[0], dtype=np.int64)
    dst = np.asarray(edge_index[1], dtype=np.int64)
    loops = np.arange(n_nodes, dtype=np.int64)
    src = np.concatenate([src, loops])
    dst = np.concatenate([dst, loops])

    deg = np.bincount(dst, minlength=n_nodes).astype(np.int64)   # >= 1
    node_order = np.argsort(-deg, kind="stable")                 # desc degree
    node_rank = np.empty(n_nodes, dtype=np.int64)
    node_rank[node_order] = np.arange(n_nodes)

    n_groups = math.ceil(n_nodes / P)
    n_groups = math.ceil(n_groups / n_cores) * n_cores           # e.g. 392
    G = n_groups // n_cores
    NROWS = n_groups * P
    PADROW = NROWS

    # edges sorted by dst; per-dst CSR and within-dst counter
    order_e = np.argsort(dst, kind="stable")
    src_s = src[order_e]
    dst_s = dst[order_e]
    csr = np.zeros(n_nodes + 1, dtype=np.int64)
    np.cumsum(deg, out=csr[1:])
    t_of_edge = np.arange(len(dst_s)) - csr[dst_s]
    # self-loop (appended last per dst segment by the stable sort) gets
    # slot 0: its source row is the node's own contiguous row, fetched by a
    # regular strided DMA instead of an indirect gather
    is_loop = order_e >= len(dst) - n_nodes
    t_of_edge = np.where(is_loop, 0, t_of_edge + 1)

    # per-rank-group T; slot j serves ranks [8j, 8j+8) => T_slot = Tg[8j]
    Tg = np.ones(n_groups, dtype=np.int64)
    for k in range(n_groups):
        if k * P < n_nodes:
            Tg[k] = deg[node_order[k * P]]
    T_slot = Tg[n_cores * np.arange(G)].astype(np.int64)
    offs = np.zeros(G + 1, dtype=np.int64)
    np.cumsum(T_slot, out=offs[1:])
    sumT = int(offs[-1])

    # snake deal: slot j, core c -> rank 8j + (c if j even else 7-c)
    rank_core = np.empty(n_groups, dtype=np.int64)
    rank_slot = np.empty(n_groups, dtype=np.int64)
    for j in range(G):
        for c in range(n_cores):
            r = n_cores * j + (c if j % 2 == 0 else n_cores - 1 - c)
            rank_core[r] = c
            rank_slot[r] = j

    # per-node placement
    n_core = rank_core[node_rank // P]          # owning core of each node
    n_slot = rank_slot[node_rank // P]          # group slot on that core
    n_part = node_rank % P                      # partition within group

    # per-core node permutation: own groups first (slot-major), then the rest
    perms = []       # perms[c][row] = node id or -1
    pos = []         # pos[c][node] = row of node in core c's ha
    node_of = []     # node_of[c][G*P] = node id or -1 (for unshard)
    for c in range(n_cores):
        perm = np.full(NROWS, -1, dtype=np.int64)
        own = n_core == c
        own_rows = n_slot[own] * P + n_part[own]
        own_nodes = np.where(own)[0]
        perm[own_rows] = own_nodes
        rest = np.where(~own)[0]
        perm[G * P:G * P + len(rest)] = rest
        p = np.empty(n_nodes, dtype=np.int64)
        rows = np.where(perm >= 0)[0]
        p[perm[rows]] = rows
        perms.append(perm)
        pos.append(p)
        node_of.append(perm[:G * P].copy())

    # per-core gather index maps
    gidx = np.full((n_cores, P, sumT), PADROW, dtype=np.int32)
    e_core = n_core[dst_s]
    e_col = offs[n_slot[dst_s]] + t_of_edge
    e_part = n_part[dst_s]
    for c in range(n_cores):
        m = e_core == c
        gidx[c, e_part[m], e_col[m]] = pos[c][src_s[m]].astype(np.int32)

    return dict(G=G, T_slot=T_slot.tolist(), offs=offs, sumT=sumT,
                NROWS=NROWS, PADROW=PADROW, perms=perms, node_of=node_of,
                gidx=gidx)


def host_constants(W, att_src, att_dst, bias):
    W = np.asarray(W, dtype=np.float32)          # [128, 256]
    att_src = np.asarray(att_src, dtype=np.float32)
    att_dst = np.asarray(att_dst, dtype=np.float32)
    Wr = W.reshape(P, HEADS, OUT_C)
    wcat = np.zeros((P, ROW), dtype=np.float32)
    wcat[:, :HC] = Wr.transpose(0, 2, 1).reshape(P, HC)       # col = c*8+h
    wcat[:, HC:HC + HEADS] = np.einsum("khc,hc->kh", Wr, att_src)
    wcat[:, HC + HEADS:] = np.einsum("khc,hc->kh", Wr, att_dst)
    padrow = np.zeros((1, ROW), dtype=np.float16)
    padrow[0, HC:HC + HEADS] = PAD_ASRC
    ident = np.eye(P, dtype=np.float16)
    bias_rep = np.tile(np.asarray(bias, dtype=np.float32)[None, :], (P, 1))
    return dict(wcat=wcat.astype(np.float16), padrow=padrow, ident=ident,
                bias_rep=bias_rep)


# ----------------------------------------------------------------------------
# Bass program (shared by all cores)
# ----------------------------------------------------------------------------

def build_bass(G: int, T_slot, sumT: int, NROWS: int):
    nc = bass.Bass(trn_type="TRN2", dynamic_dma_scratch_size=131072)

    xt_d = nc.dram_tensor("xt", [P, NROWS], F16, kind="ExternalInput")
    wcat_d = nc.dram_tensor("wcat", [P, ROW], F16, kind="ExternalInput")
    pad_d = nc.dram_tensor("padrow", [1, ROW], F16, kind="ExternalInput")
    ident_d = nc.dram_tensor("ident", [P, P], F16, kind="ExternalInput")
    bias_d = nc.dram_tensor("bias_rep", [P, OUT_C], F32, kind="ExternalInput")
    gidx_d = nc.dram_tensor("gidx", [P, sumT], I32, kind="ExternalInput")
    out_d = nc.dram_tensor("out", [G * P, OUT_C], F32, kind="ExternalOutput")

    ha_d = nc.dram_tensor("ha", [NROWS + 1, ROW], F16)   # +1 pad row

    n_tiles = NROWS // P
    n_chunks = n_tiles // CHUNK

    # ---------------- Phase A: ha = [x@W | x@WAs | x@WAd] (fp16) ------------
    with tile.TileContext(nc) as tc:
        with (
            tc.tile_pool(name="aconst", bufs=1) as cpool,
            tc.tile_pool(name="asb", bufs=3) as spool,
            tc.tile_pool(name="aps", bufs=2, space="PSUM") as ppool,
        ):
            wcat_sb = cpool.tile([P, ROW], F16, name="wcat_sb")
            nc.sync.dma_start(out=wcat_sb[:], in_=wcat_d[:, :])
            pad_sb = cpool.tile([1, ROW], F16, name="pad_sb")
            nc.sync.dma_start(out=pad_sb[:], in_=pad_d[:, :])
            nc.sync.dma_start(out=ha_d[NROWS:NROWS + 1, :], in_=pad_sb[:])

            for i in range(n_chunks):
                c0 = i * CHUNK * P
                xt = spool.tile([P, CHUNK * P], F16, tag="xt")
                nc.sync.dma_start(out=xt[:], in_=xt_d[:, c0:c0 + CHUNK * P])
                stage = spool.tile([P, CHUNK, ROW], F16, tag="stage")
                # bank-aligned PSUM (512 f32 = 1 bank per tile) so one wide
                # strided copy drains all CHUNK matmul outputs
                hps = ppool.tile([P, CHUNK, 512], F32, space="PSUM",
                                 tag="hps")
                for j in range(CHUNK):
                    nc.tensor.matmul(out=hps[:, j, 0:ROW],
                                     lhsT=xt[:, j * P:(j + 1) * P],
                                     rhs=wcat_sb[:], start=True, stop=True)
                if i % 2 == 0:
                    nc.vector.tensor_copy(out=stage[:], in_=hps[:, :, 0:ROW])
                else:
                    nc.scalar.activation(
                        out=stage[:], in_=hps[:, :, 0:ROW],
                        func=mybir.ActivationFunctionType.Copy)
                eng = nc.sync if i % 2 == 0 else nc.scalar
                eng.dma_start(
                    out=ha_d[c0:c0 + CHUNK * P, :].rearrange(
                        "(j p) c -> p j c", p=P),
                    in_=stage[:],
                )

    # ---------------- Phase B: per-group edge aggregation -------------------
    with tile.TileContext(nc) as tc:
        with (
            tc.tile_pool(name="bconst", bufs=1) as cpool,
            tc.tile_pool(name="bgath", bufs=3) as gpool,
            tc.tile_pool(name="bsmall", bufs=4) as spool,
            tc.tile_pool(name="bout", bufs=3) as opool,
            tc.tile_pool(name="bps", bufs=4, space="PSUM") as ppool,
        ):
            ident_sb = cpool.tile([P, P], F16, name="ident_sb")
            nc.sync.dma_start(out=ident_sb[:], in_=ident_d[:, :])
            bias_sb = cpool.tile([P, OUT_C], F32, name="bias_sb")
            nc.sync.dma_start(out=bias_sb[:], in_=bias_d[:, :])
            gidx_sb = cpool.tile([P, sumT], I32, name="gidx_sb")
            nc.sync.dma_start(out=gidx_sb[:], in_=gidx_d[:, :])
            shift_sb = cpool.tile([P, 1], F32, name="shift_sb")
            nc.vector.memset(shift_sb[:], EXP_SHIFT)


            off = 0
            for g in range(G):
                T = int(T_slot[g])
                gath = gpool.tile([P, T, GROW], F16, tag="gath")
                # slot 0 is every node's self-loop: contiguous rows, one
                # regular strided DMA instead of an indirect gather
                nc.sync.dma_start(out=gath[:, 0, :],
                                  in_=ha_d[g * P:(g + 1) * P, 0:GROW])
                # remaining slots: [P,1]-offset indirect gathers (the walrus
                # lowering only handles one offset per partition)
                for t0 in range(1, T, GCHUNK):
                    t1 = min(t0 + GCHUNK, T)
                    nc.gpsimd.indirect_dma_start(
                        out=gath[:, t0:t1, :].rearrange("p t c -> p (t c)"),
                        out_offset=None,
                        in_=ha_d[:, :],
                        in_offset=bass.IndirectOffsetOnAxis(
                            ap=gidx_sb[:, off + t0:off + t1], axis=0),
                    )
                adst = spool.tile([P, HEADS], F16, tag="adst")
                nc.sync.dma_start(out=adst[:],
                                  in_=ha_d[g * P:(g + 1) * P, HC + HEADS:ROW])

                att = gath[:, :, HC:GROW]
                # s = a_src + a_dst ; leaky = max(s, 0.2*s) ; ex = exp(leaky-8)
                nc.vector.tensor_tensor(
                    out=att, in0=att,
                    in1=adst[:].unsqueeze(1).to_broadcast([P, T, HEADS]),
                    op=mybir.AluOpType.add)
                sl = spool.tile([P, T, HEADS], F16, tag="sl")
                nc.scalar.activation(out=sl[:], in_=att,
                                     func=mybir.ActivationFunctionType.Copy,
                                     scale=NEG_SLOPE)
                nc.vector.tensor_tensor(out=att, in0=att, in1=sl[:],
                                        op=mybir.AluOpType.max)
                nc.scalar.activation(out=att, in_=att,
                                     func=mybir.ActivationFunctionType.Exp,
                                     bias=shift_sb[:])
                # msg = h * alpha_unnorm (head-minor keeps broadcast packed)
                nc.vector.tensor_tensor(
                    out=gath[:, :, 0:HC].rearrange("p t (c h) -> p t c h",
                                                   h=HEADS),
                    in0=gath[:, :, 0:HC].rearrange("p t (c h) -> p t c h",
                                                   h=HEADS),
                    in1=att.unsqueeze(2).to_broadcast([P, T, OUT_C, HEADS]),
                    op=mybir.AluOpType.mult)

                ops = ppool.tile([P, GROW], F32, space="PSUM", tag="ops")
                for t in range(T):
                    nc.tensor.matmul(out=ops[:], lhsT=ident_sb[:],
                                     rhs=gath[:, t, :],
                                     start=(t == 0), stop=(t == T - 1))

                # normalize + head mean + bias
                dr = spool.tile([P, HEADS], F32, tag="dr")
                nc.scalar.activation(out=dr[:], in_=ops[:, HC:GROW],
                                     func=mybir.ActivationFunctionType.Copy,
                                     scale=float(HEADS), bias=DENOM_EPS)
                rcp = spool.tile([P, HEADS], F32, tag="rcp")
                nc.vector.reciprocal(out=rcp[:], in_=dr[:])
                on = opool.tile([P, OUT_C, HEADS], F16, tag="on")
                nc.vector.tensor_tensor(
                    out=on[:],
                    in0=ops[:, 0:HC].rearrange("p (c h) -> p c h", h=HEADS),
                    in1=rcp[:].unsqueeze(1).to_broadcast([P, OUT_C, HEADS]),
                    op=mybir.AluOpType.mult)
                t1 = spool.tile([P, OUT_C, 4], F16, tag="t1")
                nc.vector.tensor_tensor(out=t1[:], in0=on[:, :, 0:4],
                                        in1=on[:, :, 4:8],
                                        op=mybir.AluOpType.add)
                t2 = spool.tile([P, OUT_C, 2], F16, tag="t2")
                nc.vector.tensor_tensor(out=t2[:], in0=t1[:, :, 0:2],
                                        in1=t1[:, :, 2:4],
                                        op=mybir.AluOpType.add)
                t3 = spool.tile([P, OUT_C], F16, tag="t3")
                nc.vector.tensor_tensor(out=t3[:],
                                        in0=t2[:, :, 0:1].rearrange(
                                            "p c one -> p (c one)"),
                                        in1=t2[:, :, 1:2].rearrange(
                                            "p c one -> p (c one)"),
                                        op=mybir.AluOpType.add)
                ob = opool.tile([P, OUT_C], F32, tag="ob")
                nc.vector.tensor_tensor(out=ob[:], in0=t3[:], in1=bias_sb[:],
                                        op=mybir.AluOpType.add)
                nc.sync.dma_start(out=out_d[g * P:(g + 1) * P, :], in_=ob[:])
                off += T

    return nc


# ----------------------------------------------------------------------------
# Walrus in this container accepts at most ONE semaphore wait per engine
# instruction.  Rebuild blocks, hoisting extra waits onto NOP carriers
# placed immediately before the instruction (same engine) — semantically
# identical (the engine just stalls one instruction earlier).
# ----------------------------------------------------------------------------

def _engine_obj(nc, engine):
    return {
        mb.EngineType.PE: nc.tensor,
        mb.EngineType.DVE: nc.vector,
        mb.EngineType.Activation: nc.scalar,
        mb.EngineType.SP: nc.sync,
        mb.EngineType.Pool: nc.gpsimd,
    }[engine]


def legalize_waits(nc, max_waits=1):
    Op = nc.isa.Opcode
    for f in nc.m.functions:
        new_blocks = []
        for blk in f.blocks:
            out = []
            for inst in blk.instructions:
                si = inst.sync_info
                waits = list(si.on_wait) if si is not None else []
                if len(waits) > max_waits:
                    eng = _engine_obj(nc, inst.engine)
                    extra, keep = waits[:-max_waits], waits[-max_waits:]
                    opc = (Op.NEURON_ISA_TPB_OPCODE_ENGINE_NOP
                           if inst.engine == mb.EngineType.Pool
                           else Op.NEURON_ISA_TPB_OPCODE_NOP)
                    for w in extra:
                        nop = eng._isa(opc, {})
                        nop.sync_info = mb.SyncInfo(on_wait=[w], on_update=[])
                        out.append(nop)
                    inst.sync_info = mb.SyncInfo(on_wait=keep,
                                                 on_update=list(si.on_update))
                out.append(inst)
            new_blocks.append(mb.BasicBlock(
                name=blk.name, instructions=out,
                IsPredicated=blk.IsPredicated, IsExit=blk.IsExit,
                IsLoopEntry=blk.IsLoopEntry))
        f.blocks = new_blocks
    return nc


# ----------------------------------------------------------------------------
# Full kernel: host prep -> run on 8 cores -> unshard
# ----------------------------------------------------------------------------

def make_in_maps(x, plan, consts, n_cores=N_CORES):
    x16 = np.asarray(x, dtype=np.float16)
    n = x16.shape[0]
    NROWS = plan["NROWS"]
    in_maps = []
    for c in range(n_cores):
        perm = plan["perms"][c]
        xp = np.zeros((NROWS, P), dtype=np.float16)
        rows = np.where(perm >= 0)[0]
        xp[rows] = x16[perm[rows]]
        m = dict(xt=np.ascontiguousarray(xp.T),
                 wcat=consts["wcat"], padrow=consts["padrow"],
                 ident=consts["ident"], bias_rep=consts["bias_rep"],
                 gidx=plan["gidx"][c])
        in_maps.append(m)
    return in_maps


def unshard(results, plan, n_nodes):
    out = np.zeros((n_nodes, OUT_C), dtype=np.float32)
    for c in range(N_CORES):
        no = plan["node_of"][c]
        res = results[c]["out"]
        mask = no >= 0
        out[no[mask]] = res[mask]
    return out


_CACHE = {}


def kernel(x, edge_index, batch, W, att_src, att_dst, bias):
    x = np.ascontiguousarray(np.asarray(x, dtype=np.float32))
    n_nodes = x.shape[0]
    plan = plan_dst(np.asarray(edge_index), n_nodes)
    key = (n_nodes, plan["G"], tuple(plan["T_slot"]), plan["NROWS"])
    if key not in _CACHE:
        nc = build_bass(plan["G"], plan["T_slot"], plan["sumT"],
                        plan["NROWS"])
        legalize_waits(nc)
        _CACHE[key] = nc
    nc = _CACHE[key]
    consts = host_constants(W, att_src, att_dst, bias)
    in_maps = make_in_maps(x, plan, consts)
    from concourse.bass_utils import run_bass_kernel_spmd
    res = run_bass_kernel_spmd(nc, in_maps, list(range(N_CORES)), trace=False)
    return unshard(res.results, plan, n_nodes)
